# revision 37
# baseline (speedup 1.0000x reference)
"""Distributed Trainium2 kernel for pre-LN multi-head self-attention.

Reference computation (n=2048, d=1024, 16 heads x 64):
    xn  = LayerNorm(x) * ln_scale + ln_bias
    qkv = xn @ w_qkv ; split -> q,k,v [16, 2048, 64]
    sim = (q @ k^T) * d**-0.5 ; attn = softmax(sim)
    out = concat_heads(attn @ v) @ w_out + b_out

Sharding: 2 heads per core (tensor parallel). Each core:
  - computes LayerNorm(x) (replicated) and xn^T via PE transposes
  - projects its 2 heads' q/k/v (ln_scale folded into weights on host,
    ln_bias folded into a per-output-column bias added at PSUM evacuation)
  - attention in transposed layout (keys on partitions) so no transposes
    are needed between the two attention matmuls; a ones-column in v
    yields softmax denominators for free
  - ONE AllGather per row chunk carrying both heads' outputs [128, w]
  - computes a 128-column slice of the final projection (+ bias)
Host assembles the 8 [128, 2048] outT shards into the [2048, 1024] output.

Scheduling notes: engines execute in emission order. The dominant
resource in the back half is the single CC stream: the five AllGathers
are bandwidth-bound (~10-20us each, serial), so row chunk 0's ENTIRE
attention (sim/exp/attn@v/normalize/AllGather) is fused into the
LayerNorm/QKV phase, block by block as K and V rows become available.
That launches AG(0) ~90us earlier and keeps the PE near-100% busy (which
also holds the HAM clock at full rate). A tiny dummy AllGather right
after startup absorbs the multi-core start skew so it does not poison
the first real collective. In the attention loop, stage s's attn@v
matmuls lag stage s+1's sim matmuls by AV_LAG key chunks so the
denominator/normalize chain never blocks the PE queue; denominators are
broadcast via a ones-matmul first, then inverted with
reciprocal_approx_fast on 64 partitions at a time. Projection for a row
chunk is deferred until its AllGather has fully drained.
"""

import sys

import ml_dtypes
import numpy as np

for _p in ("/opt/trn_rl_repo", "/root/.axon_site/_ro/trn_rl_repo"):
    if _p not in sys.path:
        sys.path.append(_p)

N = 2048          # sequence length
D = 1024          # model dim
HEADS = 16
DH = 64
NCORES = 8
HL = HEADS // NCORES          # heads per core (2)
HC = HL * DH                  # head cols per core (128)
LN_EPS = 1e-6
SIM_SCALE = float(D) ** -0.5  # reference scales by input dim

P = 128
RT = N // P        # 16 row tiles
DC = D // P        # 8 dim chunks
RC_W = 512         # max row-chunk width for attention pipeline

# Row-chunk widths: 512-wide main stages for ACT (exp) efficiency, a
# small final chunk so the last (unoverlappable) AllGather tail is short.
CHUNKS = [(0, 512), (512, 512), (1024, 512), (1536, 384), (1920, 128)]
AV_LAG = 3         # attn@v lags sim by this many key chunks (same stage)

MM_DT = "bf16"

_BUILT = None


def _build():
    """Build the SPMD Bass graph (same graph on all 8 cores)."""
    from contextlib import ExitStack

    import concourse.tile as tile
    from concourse import bacc, mybir
    from concourse.masks import make_identity

    f32 = mybir.dt.float32
    dt_mm = {"f32": f32, "f32r": mybir.dt.float32r,
             "bf16": mybir.dt.bfloat16}[MM_DT]
    AF = mybir.ActivationFunctionType

    nc = bacc.Bacc(None, num_devices=NCORES)

    x_d = nc.declare_dram_parameter("x", [N, D], f32, isOutput=False)
    wq_d = nc.declare_dram_parameter("wq", [D, HC], dt_mm, isOutput=False)
    wk_d = nc.declare_dram_parameter("wk", [D, HC], dt_mm, isOutput=False)
    wv_d = nc.declare_dram_parameter("wv", [D, HC], dt_mm, isOutput=False)
    qb_d = nc.declare_dram_parameter("qb", [HC], f32, isOutput=False)
    kb_d = nc.declare_dram_parameter("kb", [HC], f32, isOutput=False)
    vb_d = nc.declare_dram_parameter("vb", [HC], f32, isOutput=False)
    wo_d = nc.declare_dram_parameter("wo", [D, HC], dt_mm, isOutput=False)
    bo_d = nc.declare_dram_parameter("bo", [HC], f32, isOutput=False)
    out_d = nc.declare_dram_parameter("out", [HC, N], f32, isOutput=True)

    groups = [list(range(NCORES))]
    S = len(CHUNKS)

    with ExitStack() as ctx:
        tc = ctx.enter_context(tile.TileContext(nc))

        dram = ctx.enter_context(tc.tile_pool(name="dram", bufs=1, space="DRAM"))
        ag_in = [dram.tile([P, w], dt_mm, name=f"ag_in{i}")
                 for i, (_, w) in enumerate(CHUNKS)]
        ag_out = [dram.tile([NCORES * P, w], dt_mm, addr_space="Shared",
                            name=f"ag_out{i}")
                  for i, (_, w) in enumerate(CHUNKS)]
        warm_in = dram.tile([1, 16], dt_mm, name="warm_in")
        warm_out = dram.tile([NCORES, 16], dt_mm, addr_space="Shared",
                             name="warm_out")

        singles = ctx.enter_context(tc.tile_pool(name="singles", bufs=1))

        ident = singles.tile([P, P], dt_mm)
        make_identity(nc, ident)
        warm_rhs = singles.tile([P, RC_W], dt_mm)
        nc.vector.memset(warm_rhs, 0.0)
        eps_t = singles.tile([P, 1], f32)
        nc.vector.memset(eps_t, LN_EPS)
        ones_t = singles.tile([P, DH], dt_mm)
        nc.vector.memset(ones_t, 1.0)
        # dummy exp pulls the one-time ACT table load out of the hot loop
        nc.scalar.activation(out=eps_t, in_=eps_t, func=AF.Exp, scale=0.0)
        nc.vector.memset(eps_t, LN_EPS)

        # x tiles first in the DMA queue: the first LN tile gates the PE.
        xp = ctx.enter_context(tc.tile_pool(name="xp", bufs=4))
        x_tiles = []

        def emit_x_dma():
            nxt = len(x_tiles)
            if nxt < RT:
                x_t = xp.tile([P, D], f32, tag="x", name=f"x{nxt}")
                nc.sync.dma_start(out=x_t, in_=x_d[nxt * P:(nxt + 1) * P, :])
                x_tiles.append(x_t)

        for _ in range(3):
            emit_x_dma()

        # tiny collective to absorb multi-core start skew early
        wi_sb = singles.tile([1, 16], dt_mm)
        nc.vector.memset(wi_sb, 0.0)
        nc.sync.dma_start(out=warm_in[:, :], in_=wi_sb)
        nc.gpsimd.collective_compute(
            "AllGather", mybir.AluOpType.bypass, replica_groups=groups,
            ins=[warm_in[:].opt()], outs=[warm_out[:].opt()],
        )

        # weights / biases (needed from the first QKV matmul on)
        wq_sb = singles.tile([P, DC, HC], dt_mm)
        wk_sb = singles.tile([P, DC, HC], dt_mm)
        wv_sb = singles.tile([P, DC, HC], dt_mm)
        wo_sb = singles.tile([P, DC, HC], dt_mm)
        for w_sb, w_d in ((wk_sb, wk_d), (wq_sb, wq_d), (wv_sb, wv_d)):
            nc.sync.dma_start(
                out=w_sb, in_=w_d[:, :].rearrange("(c p) m -> p c m", p=P)
            )
        qb_t = singles.tile([P, 1], f32)
        kb_t = singles.tile([P, 1], f32)
        vb_t = singles.tile([P, 1], f32)
        bo_t = singles.tile([P, 1], f32)
        for b_t, b_d in ((kb_t, kb_d), (qb_t, qb_d), (vb_t, vb_d)):
            nc.sync.dma_start(out=b_t, in_=b_d[:].rearrange("(p o) -> p o", o=1))
        emit_x_dma()
        nc.sync.dma_start(
            out=wo_sb, in_=wo_d[:, :].rearrange("(c p) m -> p c m", p=P)
        )
        nc.sync.dma_start(out=bo_t, in_=bo_d[:].rearrange("(p o) -> p o", o=1))

        # long-lived activations
        qT = singles.tile([P, N], dt_mm)        # [2*64 qdims, rows]
        kT = singles.tile([P, N], dt_mm)
        v_sb = singles.tile([P, RT, HL, DH + 1], dt_mm)  # [keys, rt, h, v|1]
        attn_h = [singles.tile([DH, N], dt_mm, name=f"attn_h{h}")
                  for h in range(HL)]
        outT = singles.tile([P, N], f32)

        nc.gpsimd.memset(v_sb[:, :, :, DH:], 1.0)  # ones column

        expp = ctx.enter_context(tc.tile_pool(name="expp", bufs=2))
        rbp = ctx.enter_context(tc.tile_pool(name="rbp", bufs=2))
        dnp = ctx.enter_context(tc.tile_pool(name="dnp", bufs=2))
        agp = ctx.enter_context(tc.tile_pool(name="agp", bufs=2))
        op = ctx.enter_context(tc.tile_pool(name="op", bufs=1, space="PSUM"))

        state = {}

        def new_stage(idx):
            state[idx] = {
                "exp_t": expp.tile([P, RT, HL, RC_W], dt_mm, tag="exp",
                                   name=f"exp{idx}"),
                "po": None,
            }

        def sim_group(idx, kc, pool):
            """Both heads' sim for one key chunk, row-group packed."""
            r0, w = CHUNKS[idx]
            st = state[idx]
            ps = pool.tile([P, HL, RC_W], f32, tag="ps", name=f"ps{idx}_{kc}")
            for h in range(HL):
                nc.tensor.matmul(
                    ps[:, h, 0:w],
                    kT[h * DH:(h + 1) * DH, kc * P:(kc + 1) * P],
                    qT[h * DH:(h + 1) * DH, r0:r0 + w],
                    start=True, stop=True,
                )
            nc.scalar.activation(
                out=st["exp_t"][:, kc, :, 0:w],
                in_=ps[:, :, 0:w],
                func=AF.Exp, scale=SIM_SCALE,
            )

        def av_pair(idx, kc):
            """attn@v for key chunk kc, both heads (alternating banks)."""
            r0, w = CHUNKS[idx]
            st = state[idx]
            if st["po"] is None:
                st["po"] = op.tile([P, HL, RC_W], f32, tag="po",
                                   name=f"po{idx}")
            po = st["po"]
            for h in range(HL):
                nc.tensor.matmul(
                    po[0:DH + 1, h, 0:w],
                    v_sb[:, kc, h, :],
                    st["exp_t"][:, kc, h, 0:w],
                    start=(kc == 0), stop=(kc == RT - 1),
                )

        agt_tiles = {}

        def norm_tail(idx, pr_pool):
            """Normalize by softmax denominators, ship to the AG buffer."""
            r0, w = CHUNKS[idx]
            st = state[idx]
            po = st["po"]
            # raw denominators to SBUF (matmul rhs must be SBUF)
            dn = dnp.tile([P, HL, RC_W], dt_mm, tag="dn", name=f"dn{idx}")
            with nc.allow_low_precision(reason="softmax denoms"):
                nc.scalar.copy(out=dn[DH:DH + 1, 0, 0:w],
                               in_=po[DH:DH + 1, 0, 0:w])
                nc.vector.tensor_copy(out=dn[DH:DH + 1, 1, 0:w],
                                      in_=po[DH:DH + 1, 1, 0:w])
            # broadcast across 64 partitions via ones-matmul, then a fast
            # approximate reciprocal
            prs = [pr_pool.tile([P, RC_W], f32, tag="pm",
                                name=f"pr{idx}_{h}") for h in range(HL)]
            for h in range(HL):
                nc.tensor.matmul(
                    prs[h][0:DH, 0:w], ones_t[DH:DH + 1, :],
                    dn[DH:DH + 1, h, 0:w], start=True, stop=True,
                )
            rb = rbp.tile([DH, HL, RC_W], f32, tag="rb", name=f"rb{idx}")
            for h in range(HL):
                nc.vector.reciprocal_approx_fast(
                    out=rb[:, h, 0:w], in_=prs[h][0:DH, 0:w]
                )
            with nc.allow_low_precision(reason="attn bf16 wire"):
                for h in range(HL):
                    nc.vector.tensor_mul(
                        out=attn_h[h][:, r0:r0 + w],
                        in0=po[0:DH, h, 0:w], in1=rb[:, h, 0:w],
                    )
            for h in range(HL):
                nc.gpsimd.dma_start(
                    out=ag_in[idx][h * DH:(h + 1) * DH, :],
                    in_=attn_h[h][:, r0:r0 + w],
                )
            nc.gpsimd.collective_compute(
                "AllGather",
                mybir.AluOpType.bypass,
                replica_groups=groups,
                ins=[ag_in[idx][:].opt()],
                outs=[ag_out[idx][:].opt()],
            )
            # gather-in DMA on the (idle) sync queue: it waits on the AG
            # semaphore there without blocking the gpsimd collective queue,
            # so the transfer starts the instant the AllGather completes.
            agt = agp.tile([P, DC, RC_W], dt_mm, tag="agt", name=f"agt{idx}")
            nc.sync.dma_start(
                out=agt[:, :, 0:w],
                in_=ag_out[idx][:, :].rearrange("(c p) m -> p c m", p=P),
            )
            agt_tiles[idx] = agt

        def proj(idx, pf_pool):
            """outT slice for this row chunk from the gathered heads."""
            r0, w = CHUNKS[idx]
            agt = agt_tiles.pop(idx)
            pf = pf_pool.tile([P, RC_W], f32, tag="pm", name=f"pf{idx}")
            for kc in range(DC):
                nc.tensor.matmul(
                    pf[:, 0:w],
                    wo_sb[:, kc, :],
                    agt[:, kc, 0:w],
                    start=(kc == 0), stop=(kc == DC - 1),
                )
            nc.vector.tensor_scalar(
                out=outT[:, r0:r0 + w], in0=pf[:, 0:w],
                scalar1=bo_t, scalar2=None,
                op0=mybir.AluOpType.add,
            )
            nc.gpsimd.dma_start(
                out=out_d[:, r0:r0 + w], in_=outT[:, r0:r0 + w]
            )

        # ---- phase 1: LayerNorm -> xn^T -> q/k/v, fused with the ENTIRE
        # attention of row chunk 0 (its sims/avs consume each block's K/V
        # right after they are produced, so AG(0) launches at phase end) ----
        new_stage(0)
        with (
            tc.tile_pool(name="stat", bufs=6) as statp,
            tc.tile_pool(name="tp", bufs=2, space="PSUM") as tp,
            tc.tile_pool(name="mmp", bufs=2, space="PSUM") as mmp,
            tc.tile_pool(name="sp0", bufs=1, space="PSUM") as sp0,
            tc.tile_pool(name="xnTp", bufs=1) as xnTp,
        ):
            xnT = xnTp.tile([P, DC, N], dt_mm)   # [dim%128, dimchunk, rows]
            vT = xnTp.tile([P, N], dt_mm)

            # dependency-free matmul burst warms the HAM clock while the
            # first x tile lands
            warm_ps = mmp.tile([P, 512], f32, tag="pm", name="warm")
            for _ in range(8):
                nc.tensor.matmul(warm_ps, ident, warm_rhs,
                                 start=True, stop=True)

            def s0_work(g4, j):
                """One stage-0 sim+av consuming block g4-1's K/V; attn@v
                lags by one block so the PE never waits on the exp."""
                if g4 == 0:
                    return
                kc = (g4 - 1) * 4 + j
                sim_group(0, kc, sp0)
                if kc >= 4:
                    av_pair(0, kc - 4)

            for g4 in range(RT // 4):
                # LN for the block's 4 row tiles
                xh_list = []
                for rt in range(g4 * 4, g4 * 4 + 4):
                    x_t = x_tiles[rt]
                    emit_x_dma()
                    stats = statp.tile([P, 2, 6], f32, tag="st")
                    for sg in range(2):
                        nc.vector.bn_stats(
                            out=stats[:, sg, :],
                            in_=x_t[:, sg * 512:(sg + 1) * 512],
                        )
                    mv = statp.tile([P, 2], f32, tag="mv")
                    nc.vector.bn_aggr(out=mv, in_=stats)
                    rstd = statp.tile([P, 1], f32, tag="rstd")
                    nc.scalar.activation(
                        out=rstd, in_=mv[:, 1:2], func=AF.Sqrt,
                        bias=eps_t, scale=1.0,
                    )
                    nc.vector.reciprocal(out=rstd, in_=rstd)
                    xh_t = xp.tile([P, D], dt_mm, tag="xh")
                    nc.vector.tensor_scalar(
                        out=xh_t, in0=x_t,
                        scalar1=mv[:, 0:1], scalar2=rstd,
                        op0=mybir.AluOpType.subtract, op1=mybir.AluOpType.mult,
                    )
                    xh_list.append(xh_t)

                # transposes: one PSUM tile per dim-chunk spanning the 4 row
                # tiles -> contiguous [128, 512] evacuations (DVE/ACT split)
                for dc in range(DC):
                    pt = tp.tile([P, 512], dt_mm, tag="pt")
                    with nc.allow_low_precision(reason="transpose copy"):
                        for j in range(4):
                            nc.tensor.transpose(
                                pt[:, j * P:(j + 1) * P],
                                xh_list[j][:, dc * P:(dc + 1) * P],
                                ident,
                            )
                    dst = xnT[:, dc, g4 * 512:(g4 + 1) * 512]
                    if dc == 0:
                        nc.vector.tensor_copy(out=dst, in_=pt)
                    else:
                        nc.scalar.copy(out=dst, in_=pt)
                    if dc == 3:
                        for j in range(4):
                            s0_work(g4, j)  # interleave stage-0 attention

                # q/k/v projections for this 512-row block
                nt = g4
                for w_sb, b_t, dst in (
                    (wk_sb, kb_t, kT), (wv_sb, vb_t, vT), (wq_sb, qb_t, qT)
                ):
                    pm = mmp.tile([P, 512], f32, tag="pm")
                    for kc in range(DC):
                        nc.tensor.matmul(
                            pm,
                            w_sb[:, kc, :],
                            xnT[:, kc, nt * 512:(nt + 1) * 512],
                            start=(kc == 0), stop=(kc == DC - 1),
                        )
                    nc.scalar.activation(
                        out=dst[:, nt * 512:(nt + 1) * 512], in_=pm,
                        func=AF.Identity, bias=b_t, scale=1.0,
                    )
                # v^T -> v (row-major with ones column) for this block
                for rt in range(g4 * 4, g4 * 4 + 4):
                    pt = tp.tile([P, 512], dt_mm, tag="pt")
                    with nc.allow_low_precision(reason="transpose copy"):
                        nc.tensor.transpose(
                            pt[:, :P], vT[:, rt * P:(rt + 1) * P], ident
                        )
                    nc.vector.tensor_copy(
                        out=v_sb[:, rt, :, 0:DH],
                        in_=pt[:, :P].rearrange("p (h d) -> p h d", h=HL),
                    )

            # stage 0 tail: last block's key chunks + normalize + AllGather
            for kc in range(12, 16):
                sim_group(0, kc, sp0)
                av_pair(0, kc - 4)
            for kc in range(12, 16):
                av_pair(0, kc)
            norm_tail(0, mmp)

        # ---- phase 2: attention stages 1..S-1. Each stage's attn@v lags
        # its own sims by AV_LAG key chunks (the stage pace is ACT-bound
        # either way) so every AllGather launches right at stage end,
        # keeping the serial CC stream fed. proj(c) runs early in stage
        # c+2, by which point AG(c) has drained. ----
        with (
            tc.tile_pool(name="sp", bufs=2, space="PSUM") as sp,
            tc.tile_pool(name="rp", bufs=1, space="PSUM") as rp,
            tc.tile_pool(name="filp", bufs=1, space="PSUM") as filp,
        ):
            # dependency-free filler matmuls keep the PE's HAM activity
            # window "busy" through the ACT-paced attention stages so the
            # PE clock stays at full rate for the real sim/attn@v matmuls.
            fil = filp.tile([P, RC_W], f32, tag="fil", name="fil")

            def filler(n):
                for _ in range(n):
                    nc.tensor.matmul(fil, ident, warm_rhs,
                                     start=True, stop=True)

            for idx in range(1, S):
                new_stage(idx)
                for kc in range(RT):
                    sim_group(idx, kc, sp)
                    if kc >= AV_LAG:
                        av_pair(idx, kc - AV_LAG)
                    filler(2)
                    if idx >= 2 and kc == 2:
                        proj(idx - 2, rp)
                for j in range(RT - AV_LAG, RT):
                    av_pair(idx, j)
                norm_tail(idx, rp)
            # drain: remaining projections
            proj(S - 2, rp)
            proj(S - 1, rp)

    if not nc.is_finalized():
        nc.finalize()
    return nc


def _get_built():
    global _BUILT
    if _BUILT is None:
        _BUILT = _build()
    return _BUILT


def _shard_inputs(x, ln_scale, ln_bias, w_qkv, w_out, b_out):
    """Host-side sharding: slice per-head weight columns, fold LN params."""
    x = np.ascontiguousarray(np.asarray(x, np.float32))
    ln_scale = np.asarray(ln_scale, np.float32)
    ln_bias = np.asarray(ln_bias, np.float32)
    w_qkv = np.asarray(w_qkv, np.float32)
    w_out = np.asarray(w_out, np.float32)
    b_out = np.asarray(b_out, np.float32)

    w_np = {"f32": np.float32, "f32r": np.float32,
            "bf16": ml_dtypes.bfloat16}[MM_DT]

    in_maps = []
    for ci in range(NCORES):
        c0 = ci * HC
        sl = {}
        for name, off in (("q", 0), ("k", HEADS * DH), ("v", 2 * HEADS * DH)):
            w = w_qkv[:, off + c0: off + c0 + HC]
            sl["w" + name] = np.ascontiguousarray(
                (ln_scale[:, None] * w).astype(w_np)
            )
            sl[name + "b"] = np.ascontiguousarray(ln_bias @ w)
        sl["wo"] = np.ascontiguousarray(w_out[:, c0:c0 + HC].astype(w_np))
        sl["bo"] = np.ascontiguousarray(b_out[c0:c0 + HC])
        sl["x"] = x
        in_maps.append(sl)
    return in_maps


def kernel(x, ln_scale, ln_bias, w_qkv, w_out, b_out):
    from concourse.bass_utils import run_bass_kernel_spmd

    nc = _get_built()
    in_maps = _shard_inputs(x, ln_scale, ln_bias, w_qkv, w_out, b_out)
    res = run_bass_kernel_spmd(nc, in_maps, core_ids=list(range(NCORES)))
    shards = [res.results[ci]["out"] for ci in range(NCORES)]  # [128, 2048]
    outT = np.concatenate(shards, axis=0)  # [1024, 2048]
    return np.ascontiguousarray(outT.T)
